# revision 1
# baseline (speedup 1.0000x reference)
"""EntNetQA Trainium2 kernel (8-core SPMD, data-parallel over batch).

Problem shapes: B=64, Q=20, S=20, Rn=10, L=60, K=NUM_BLOCKS=20, E=256,
VOCAB=20020, READOUT=20000.

Key algorithmic observation: the recurrent scan input hist_enc is broadcast
over Q before the scan and state0 (keys) is bq-independent, so the scanned
state only depends on b. We run the scan on [B, K, E] (64 states) instead of
[B*Q, K, E] (1280) - a 20x reduction - and only the output module (attention
over q_enc + readout) runs per (b, q).

Sharding: core c owns batches b in [8c, 8c+8) => 160 of the 1280 (b,q) rows.

Per-core pipeline (one Bass program, SPMD over 8 cores):
  1. Indirect-DMA gathers of story tokens (one batched gather per round),
     segment-summed by tiny PE matmuls into hist_encT [E, (r,b)].
  2. xwT = W @ x per round.
  3. 10 sequential EntNet rounds on stateT [E=2x128, (b,k)=160]: partition
     reductions / broadcasts via ones-matmuls and K=1 outer products on PE.
  4. Query gathers -> q_encT [E, (b,q)] (overlaps the scan).
  5. Output module: blockwise attention via a masked full softmax (the
     off-diagonal b-blocks get -1e30 so attn_full IS the block-diag mixing
     matrix), u, p = prelu(q_enc + u H^T).
  6. Readout y = p @ R^T in bf16 hi/lo split-3 (p_hi R_hi + p_hi R_lo +
     p_lo R_hi; the dropped lo*lo term is ~2^-18 relative): bf16 matmuls run
     the PE at 1 cycle/row vs fp32's 4, with ~4e-6 output error. R_hi/R_lo
     are host-precomputed, streamed from HBM during the earlier phases.

Host side: pre-transposes U/W/H/R, splits R into bf16 hi+lo, folds
bias + keys@V^T into one constant, zeroes the padding embedding row, builds
segment/mask matrices, and reassembles the 8 per-core outputs.
"""

import numpy as np
from contextlib import ExitStack

import concourse.bass as bass
import concourse.tile as tile
from concourse import bacc, mybir
from concourse.masks import make_identity
from concourse.bass_utils import run_bass_kernel_spmd

F32 = mybir.dt.float32
BF16 = mybir.dt.bfloat16
I32 = mybir.dt.int32

B, Q, S, Rn, L = 64, 20, 20, 10, 60
K, E, V, RO = 20, 256, 20020, 20000
NC = 8            # cores
NB = B // NC      # batches per core = 8
BK = NB * K       # 160 state rows per core
BQc = NB * Q      # 160 (b,q) rows per core
RND_TOK = 512     # per-round gathered tokens, padded (8*60 -> 512 = 4 tiles)
H_TILES = 4       # hist gather tiles per round
Q_TILES = (BQc * S) // 128  # 3200/128 = 25
N_TILE = 512      # readout free-dim tile (one PSUM bank of fp32 out)
N_TILES = (RO + N_TILE - 1) // N_TILE  # 40


def build_program(general_mask=False, general_prelu=False):
    """Trace + compile the per-core Bass program. Same program for all cores."""
    nc = bacc.Bacc("TRN2", target_bir_lowering=False, debug=False)

    d_emb = nc.dram_tensor("emb", [V, E], F32, kind="ExternalInput")
    d_rth = nc.dram_tensor("rth", [E, RO], BF16, kind="ExternalInput")
    d_rtl = nc.dram_tensor("rtl", [E, RO], BF16, kind="ExternalInput")
    d_ut = nc.dram_tensor("ut", [E, E], F32, kind="ExternalInput")
    d_wt = nc.dram_tensor("wt", [E, E], F32, kind="ExternalInput")
    d_ht = nc.dram_tensor("ht", [E, E], F32, kind="ExternalInput")
    d_keyst = nc.dram_tensor("keyst", [E, K], F32, kind="ExternalInput")
    d_cbt = nc.dram_tensor("cbt", [E, K], F32, kind="ExternalInput")
    d_hseg = nc.dram_tensor("hseg", [H_TILES, 128, NB], F32, kind="ExternalInput")
    d_qseg = nc.dram_tensor("qseg", [Q_TILES, 128, BQc], F32, kind="ExternalInput")
    d_idxh = nc.dram_tensor("idxh", [Rn * H_TILES, 128], I32, kind="ExternalInput")
    d_idxq = nc.dram_tensor("idxq", [Q_TILES, 128], I32, kind="ExternalInput")
    d_amask = nc.dram_tensor("amask", [BQc, BK], F32, kind="ExternalInput")
    if general_mask:
        d_hmsk = nc.dram_tensor("hmsk", [H_TILES, 128, E], F32, kind="ExternalInput")
        d_qmsk = nc.dram_tensor("qmsk", [5, 128, E], F32, kind="ExternalInput")
    if general_prelu:
        d_at = nc.dram_tensor("at", [E, 1], F32, kind="ExternalInput")
    d_y = nc.dram_tensor("y", [BQc, RO], F32, kind="ExternalOutput")

    with tile.TileContext(nc) as tc, ExitStack() as ctx:
        # ---------------- SBUF pools ----------------
        consts = ctx.enter_context(tc.tile_pool(name="consts", bufs=1))
        ghist_p = ctx.enter_context(tc.tile_pool(name="ghist", bufs=3))
        gqa_p = ctx.enter_context(tc.tile_pool(name="gqa", bufs=3))
        enc_p = ctx.enter_context(tc.tile_pool(name="enc", bufs=1))
        state_p = ctx.enter_context(tc.tile_pool(name="state", bufs=2))
        scratch_p = ctx.enter_context(tc.tile_pool(name="scratch", bufs=2))
        rows_p = ctx.enter_context(tc.tile_pool(name="rows", bufs=4))
        ph3_p = ctx.enter_context(tc.tile_pool(name="ph3", bufs=1))
        rt_p = ctx.enter_context(tc.tile_pool(name="rtpool", bufs=18))
        ysb_p = ctx.enter_context(tc.tile_pool(name="ysb", bufs=3))

        # ---------------- constants to SBUF ----------------
        ut_sb = consts.tile([128, 2, E], F32, tag="ut")
        nc.sync.dma_start(ut_sb[:], d_ut.ap().rearrange("(h p) f -> p h f", p=128))
        wt_sb = consts.tile([128, 2, E], F32, tag="wt")
        nc.sync.dma_start(wt_sb[:], d_wt.ap().rearrange("(h p) f -> p h f", p=128))
        ht_sb = consts.tile([128, 2, E], F32, tag="ht")
        nc.sync.dma_start(ht_sb[:], d_ht.ap().rearrange("(h p) f -> p h f", p=128))
        keyst_sb = consts.tile([128, 2, K], F32, tag="keyst")
        nc.sync.dma_start(keyst_sb[:], d_keyst.ap().rearrange("(h p) k -> p h k", p=128))
        cbt_sb = consts.tile([128, 2, K], F32, tag="cbt")
        nc.sync.dma_start(cbt_sb[:], d_cbt.ap().rearrange("(h p) k -> p h k", p=128))
        hseg_sb = consts.tile([128, H_TILES, NB], F32, tag="hseg")
        nc.sync.dma_start(hseg_sb[:], d_hseg.ap().rearrange("t p g -> p t g"))
        qseg_sb = consts.tile([128, Q_TILES, BQc], F32, tag="qseg")
        nc.sync.dma_start(qseg_sb[:], d_qseg.ap().rearrange("t p g -> p t g"))
        idxh_sb = consts.tile([128, Rn * H_TILES], I32, tag="idxh")
        nc.sync.dma_start(idxh_sb[:], d_idxh.ap().rearrange("t p -> p t"))
        idxq_sb = consts.tile([128, Q_TILES], I32, tag="idxq")
        nc.sync.dma_start(idxq_sb[:], d_idxq.ap().rearrange("t p -> p t"))
        ones_sb = consts.tile([128, 128], F32, tag="ones")
        nc.vector.memset(ones_sb[:], 1.0)
        ident = consts.tile([128, 128], F32, tag="ident")
        make_identity(nc, ident[:])
        amask_sb = [consts.tile([80, BK], F32, tag=f"amask{g}", name=f"amask{g}")
                    for g in range(2)]
        for g in range(2):
            nc.sync.dma_start(amask_sb[g][:], d_amask.ap()[g * 80:(g + 1) * 80, :])
        if general_mask:
            hmsk_sb = consts.tile([128, H_TILES, E], F32, tag="hmsk")
            nc.sync.dma_start(hmsk_sb[:], d_hmsk.ap().rearrange("t p e -> p t e"))
            qmsk_sb = consts.tile([128, 5, E], F32, tag="qmsk")
            nc.sync.dma_start(qmsk_sb[:], d_qmsk.ap().rearrange("t p e -> p t e"))
        if general_prelu:
            at_sb = consts.tile([128, 2, 1], F32, tag="at")
            nc.sync.dma_start(at_sb[:], d_at.ap().rearrange("(h p) o -> p h o", p=128))

        # persistent encodings
        hT = enc_p.tile([128, 2, Rn * NB], F32, tag="hT")      # hist_encT [e, (r,b)]
        xwT = enc_p.tile([128, 2, Rn * NB], F32, tag="xwT")    # (x @ W.T)^T [f, (r,b)]
        qT = enc_p.tile([128, 2, BQc], F32, tag="qT")          # q_encT [e, (b,q)]
        pTh = enc_p.tile([128, 2, BQc], BF16, tag="pTh")       # readout p^T hi
        pTl = enc_p.tile([128, 2, BQc], BF16, tag="pTl")       # readout p^T lo

        emb_ap = d_emb.ap()

        def gather_block(pool, idx_sb, c0, m):
            """Gather m embedding rows per partition (indices idx_sb[:, c0:c0+m])
            in ONE indirect DMA -> [128, m, E]."""
            g = pool.tile([128, m, E], F32, tag=f"gath{m}", name=f"gath{m}")
            for t in range(m):
                nc.gpsimd.indirect_dma_start(
                    out=g[:, t, :],
                    out_offset=None,
                    in_=emb_ap,
                    in_offset=bass.IndirectOffsetOnAxis(ap=idx_sb[:, c0 + t:c0 + t + 1], axis=0),
                )
            return g

        # ================= era A: gathers + encodings + scan =================
        with tc.tile_pool(name="psumA", bufs=1, space="PSUM") as psA:
            # ---- hist gathers + segment sums + xw, per round ----
            for r in range(Rn):
                hps = psA.tile([128, 2, NB], F32, tag="hx", bufs=1, space="PSUM")
                g4 = gather_block(ghist_p, idxh_sb, r * H_TILES, H_TILES)
                if general_mask:
                    gm = ghist_p.tile([128, H_TILES, E], F32, tag="gathm")
                    nc.vector.tensor_tensor(out=gm[:], in0=g4[:], in1=hmsk_sb[:],
                                            op=mybir.AluOpType.mult)
                    g4 = gm
                for mh in range(2):
                    for t in range(H_TILES):
                        nc.tensor.matmul(
                            hps[:, mh, :],
                            lhsT=g4[:, t, mh * 128:(mh + 1) * 128],
                            rhs=hseg_sb[:, t, :],
                            start=(t == 0),
                            stop=(t == H_TILES - 1),
                        )
                nc.vector.tensor_copy(out=hT[:, :, r * NB:(r + 1) * NB], in_=hps[:])
                xps = psA.tile([128, 2, NB], F32, tag="hx", bufs=1, space="PSUM")
                for mh in range(2):
                    for kh in range(2):
                        nc.tensor.matmul(
                            xps[:, mh, :],
                            lhsT=wt_sb[:, kh, mh * 128:(mh + 1) * 128],
                            rhs=hT[:, kh, r * NB:(r + 1) * NB],
                            start=(kh == 0),
                            stop=(kh == 1),
                        )
                nc.vector.tensor_copy(out=xwT[:, :, r * NB:(r + 1) * NB], in_=xps[:])

            # ---- the scan ----
            keysB = keyst_sb[:, :, None, :].to_broadcast([128, 2, NB, K])
            cbB = cbt_sb[:, :, None, :].to_broadcast([128, 2, NB, K])
            st = state_p.tile([128, 2, BK], F32, tag="state")
            nc.vector.tensor_copy(out=st[:].rearrange("p h (b k) -> p h b k", k=K),
                                  in_=keysB)

            for r in range(Rn):
                st4 = st[:].rearrange("p h (b k) -> p h b k", k=K)
                rsl = slice(r * NB, (r + 1) * NB)
                xb = hT[:, :, rsl][:, :, :, None].to_broadcast([128, 2, NB, K])
                xwb = xwT[:, :, rsl][:, :, :, None].to_broadcast([128, 2, NB, K])

                # gate logits: sum_e (state + keys) * x
                gp = scratch_p.tile([128, 2, BK], F32, tag="gp")
                gp4 = gp[:].rearrange("p h (b k) -> p h b k", k=K)
                nc.vector.tensor_tensor(out=gp4, in0=st4, in1=keysB, op=mybir.AluOpType.add)
                nc.vector.tensor_tensor(out=gp4, in0=gp4, in1=xb, op=mybir.AluOpType.mult)
                glog_t = psA.tile([1, BK], F32, tag="rowp", bufs=2, space="PSUM")
                glog = glog_t[:]
                for kh in range(2):
                    nc.tensor.matmul(glog, lhsT=ones_sb[:, 0:1], rhs=gp[:, kh, :],
                                     start=(kh == 0), stop=(kh == 1))
                gate = rows_p.tile([1, BK], F32, tag="gate")
                nc.scalar.activation(gate[:], glog, mybir.ActivationFunctionType.Sigmoid)
                gb_t = psA.tile([128, BK], F32, tag="bcp", bufs=2, space="PSUM")
                gb = gb_t[:]
                nc.tensor.matmul(gb, lhsT=ones_sb[0:1, :], rhs=gate[:], start=True, stop=True)

                # candidate: U @ state + (xW^T + bias + keys V^T)
                cpre = scratch_p.tile([128, 2, BK], F32, tag="cpre")
                cpre4 = cpre[:].rearrange("p h (b k) -> p h b k", k=K)
                nc.vector.tensor_tensor(out=cpre4, in0=xwb, in1=cbB, op=mybir.AluOpType.add)
                cand = psA.tile([128, 2, BK], F32, tag="cand", bufs=2, space="PSUM")
                for mh in range(2):
                    for kh in range(2):
                        nc.tensor.matmul(
                            cand[:, mh, :],
                            lhsT=ut_sb[:, kh, mh * 128:(mh + 1) * 128],
                            rhs=st[:, kh, :],
                            start=(kh == 0),
                            stop=(kh == 1),
                        )
                candf = scratch_p.tile([128, 2, BK], F32, tag="candf")
                nc.vector.tensor_tensor(out=candf[:], in0=cand[:], in1=cpre[:],
                                        op=mybir.AluOpType.add)
                if general_prelu:
                    r1 = scratch_p.tile([128, 2, BK], F32, tag="prelu1")
                    nc.scalar.activation(r1[:], candf[:], mybir.ActivationFunctionType.Relu)
                    r2 = scratch_p.tile([128, 2, BK], F32, tag="prelu2")
                    nc.scalar.activation(r2[:], candf[:], mybir.ActivationFunctionType.Relu,
                                         scale=-1.0)
                    for eh in range(2):
                        nc.vector.tensor_scalar_mul(r2[:, eh, :], r2[:, eh, :], at_sb[:, eh, :])
                    nc.vector.tensor_tensor(out=candf[:], in0=r1[:], in1=r2[:],
                                            op=mybir.AluOpType.subtract)

                # s = state + gate * cand
                gbb = gb[:, None, :].to_broadcast([128, 2, BK])
                s_new = state_p.tile([128, 2, BK], F32, tag="state", name="s_new")
                gc = scratch_p.tile([128, 2, BK], F32, tag="gc")
                nc.vector.tensor_tensor(out=gc[:], in0=candf[:], in1=gbb, op=mybir.AluOpType.mult)
                nc.vector.tensor_tensor(out=s_new[:], in0=st[:], in1=gc[:], op=mybir.AluOpType.add)

                # norm = sqrt(sum_e s^2) + 1e-8 ; s = where(s>0, s, 1) / norm
                ssq = scratch_p.tile([128, 2, BK], F32, tag="ssq")
                nc.scalar.activation(ssq[:], s_new[:], mybir.ActivationFunctionType.Square)
                nsq_t = psA.tile([1, BK], F32, tag="rowp", bufs=2, space="PSUM")
                nsq = nsq_t[:]
                for kh in range(2):
                    nc.tensor.matmul(nsq, lhsT=ones_sb[:, 0:1], rhs=ssq[:, kh, :],
                                     start=(kh == 0), stop=(kh == 1))
                nr = rows_p.tile([1, BK], F32, tag="nr")
                nc.scalar.activation(nr[:], nsq, mybir.ActivationFunctionType.Sqrt)
                rinv = rows_p.tile([1, BK], F32, tag="rinv")
                nc.vector.reciprocal(rinv[:], nr[:])
                rb_t = psA.tile([128, BK], F32, tag="bcp", bufs=2, space="PSUM")
                rb = rb_t[:]
                nc.tensor.matmul(rb, lhsT=ones_sb[0:1, :], rhs=rinv[:], start=True, stop=True)

                mask = scratch_p.tile([128, 2, BK], mybir.dt.uint8, tag="mask")
                nc.vector.tensor_scalar(out=mask[:], in0=s_new[:], scalar1=0.0, scalar2=None,
                                        op0=mybir.AluOpType.is_le)
                rbb = rb[:, None, :].to_broadcast([128, 2, BK])
                nc.vector.tensor_tensor(out=s_new[:], in0=s_new[:], in1=rbb,
                                        op=mybir.AluOpType.mult)
                for eh in range(2):
                    nc.vector.copy_predicated(s_new[:, eh, :], mask[:, eh, :], rb)
                st = s_new

            # ---- query gathers, accumulate q_encT in SBUF ----
            QB = 5
            for j in range(Q_TILES // QB):
                g5 = gather_block(gqa_p, idxq_sb, j * QB, QB)
                if general_mask:
                    gm = gqa_p.tile([128, QB, E], F32, tag="gathqm")
                    nc.vector.tensor_tensor(out=gm[:], in0=g5[:], in1=qmsk_sb[:],
                                            op=mybir.AluOpType.mult)
                    g5 = gm
                qtmp = psA.tile([128, 2, BQc], F32, tag="cand", bufs=2, space="PSUM")
                for mh in range(2):
                    for tt in range(QB):
                        nc.tensor.matmul(
                            qtmp[:, mh, :],
                            lhsT=g5[:, tt, mh * 128:(mh + 1) * 128],
                            rhs=qseg_sb[:, j * QB + tt, :],
                            start=(tt == 0),
                            stop=(tt == QB - 1),
                        )
                if j == 0:
                    nc.vector.tensor_copy(out=qT[:], in_=qtmp[:])
                else:
                    nc.vector.tensor_tensor(out=qT[:], in0=qT[:], in1=qtmp[:],
                                            op=mybir.AluOpType.add)


        # ================= era B: output module + readout =================
        with tc.tile_pool(name="psumB", bufs=1, space="PSUM") as psB:
            # attention logits (full 160x160 per 80-row group) + block mask
            attn_full = []
            for gidx in range(2):
                bp = psB.tile([80, BK], F32, tag="mix", bufs=2, space="PSUM")
                for kh in range(2):
                    nc.tensor.matmul(bp[:], lhsT=qT[:, kh, gidx * 80:(gidx + 1) * 80],
                                     rhs=st[:, kh, :], start=(kh == 0), stop=(kh == 1))
                alog = ph3_p.tile([80, BK], F32, tag=f"alog{gidx}", name=f"alog{gidx}")
                nc.vector.tensor_tensor(out=alog[:], in0=bp[:], in1=amask_sb[gidx][:],
                                        op=mybir.AluOpType.add)
                negmax = ph3_p.tile([80, 1], F32, tag=f"negmax{gidx}", name=f"negmax{gidx}")
                nc.vector.tensor_reduce(out=negmax[:], in_=alog[:],
                                        axis=mybir.AxisListType.X, op=mybir.AluOpType.max,
                                        negate=True)
                ex = ph3_p.tile([80, BK], F32, tag=f"ex{gidx}", name=f"ex{gidx}")
                sumex = ph3_p.tile([80, 1], F32, tag=f"sumex{gidx}", name=f"sumex{gidx}")
                nc.scalar.activation(ex[:], alog[:], mybir.ActivationFunctionType.Exp,
                                     bias=negmax[:], accum_out=sumex[:])
                rinv2 = ph3_p.tile([80, 1], F32, tag=f"rinv2{gidx}", name=f"rinv2{gidx}")
                nc.vector.reciprocal(rinv2[:], sumex[:])
                at2 = ph3_p.tile([80, BK], F32, tag=f"att{gidx}", name=f"att{gidx}")
                nc.vector.tensor_scalar_mul(at2[:], ex[:], rinv2[:])
                attn_full.append(at2)

            # attn^T as [128, BQc] + [32, BQc] tiles
            A0 = ph3_p.tile([128, BQc], F32, tag="A0")
            A1 = ph3_p.tile([32, BQc], F32, tag="A1")
            for gidx in range(2):
                tp = psB.tile([128, 128], F32, tag="mix", bufs=2, space="PSUM")
                nc.tensor.transpose(tp[:, :80], attn_full[gidx][:, 0:128], ident[:80, :80])
                nc.vector.tensor_copy(out=A0[:, gidx * 80:(gidx + 1) * 80], in_=tp[:, :80])
                tp2 = psB.tile([128, 128], F32, tag="mix", bufs=2, space="PSUM")
                nc.tensor.transpose(tp2[:32, :80], attn_full[gidx][:, 128:BK], ident[:80, :80])
                nc.vector.tensor_copy(out=A1[:, gidx * 80:(gidx + 1) * 80], in_=tp2[:32, :80])

            # state in normal layout, [128, E] + [32, E]
            stn0 = ph3_p.tile([128, E], F32, tag="stn0")
            stn1 = ph3_p.tile([32, E], F32, tag="stn1")
            for kh in range(2):
                tp = psB.tile([128, 128], F32, tag="mix", bufs=2, space="PSUM")
                nc.tensor.transpose(tp[:], st[:, kh, 0:128], ident[:])
                nc.vector.tensor_copy(out=stn0[:, kh * 128:(kh + 1) * 128], in_=tp[:])
                tp2 = psB.tile([128, 128], F32, tag="mix", bufs=2, space="PSUM")
                nc.tensor.transpose(tp2[:32, :], st[:, kh, 128:BK], ident[:])
                nc.vector.tensor_copy(out=stn1[:, kh * 128:(kh + 1) * 128], in_=tp2[:32, :])

            # u = attn^T-matmul(stn), contraction over (b,k) split 128+32
            u_sb = [ph3_p.tile([128, E], F32, tag="usb0", name="usb0"),
                    ph3_p.tile([32, E], F32, tag="usb1", name="usb1")]
            for m2 in range(2):
                mP = 128 if m2 == 0 else BQc - 128
                up = psB.tile([128, E], F32, tag="mix", bufs=2, space="PSUM")
                msl = slice(m2 * 128, m2 * 128 + mP)
                nc.tensor.matmul(up[:mP, :], lhsT=A0[:, msl], rhs=stn0[:], start=True, stop=False)
                nc.tensor.matmul(up[:mP, :], lhsT=A1[:, msl], rhs=stn1[:], start=False, stop=True)
                nc.vector.tensor_copy(out=u_sb[m2][:], in_=up[:mP, :])

            # u^T [e, (b,q)]
            uT = ph3_p.tile([128, 2, BQc], F32, tag="uT")
            for kh in range(2):
                tp = psB.tile([128, 128], F32, tag="mix", bufs=2, space="PSUM")
                nc.tensor.transpose(tp[:], u_sb[0][:, kh * 128:(kh + 1) * 128], ident[:])
                nc.vector.tensor_copy(out=uT[:, kh, 0:128], in_=tp[:])
                tp2 = psB.tile([128, 128], F32, tag="mix", bufs=2, space="PSUM")
                nc.tensor.transpose(tp2[:, :32], u_sb[1][:, kh * 128:(kh + 1) * 128],
                                    ident[:32, :32])
                nc.vector.tensor_copy(out=uT[:, kh, 128:BK], in_=tp2[:, :32])

            # p^T = prelu(q_enc^T + H^T-matmul(u^T)); split into bf16 hi + lo
            pq = psB.tile([128, 2, BQc], F32, tag="pq", bufs=1, space="PSUM")
            for mh in range(2):
                for kh in range(2):
                    nc.tensor.matmul(
                        pq[:, mh, :],
                        lhsT=ht_sb[:, kh, mh * 128:(mh + 1) * 128],
                        rhs=uT[:, kh, :],
                        start=(kh == 0), stop=(kh == 1),
                    )
            pT = ph3_p.tile([128, 2, BQc], F32, tag="pT")
            nc.vector.tensor_tensor(out=pT[:], in0=pq[:], in1=qT[:], op=mybir.AluOpType.add)
            if general_prelu:
                r1 = ph3_p.tile([128, 2, BQc], F32, tag="prelu1p", name="prelu1p")
                nc.scalar.activation(r1[:], pT[:], mybir.ActivationFunctionType.Relu)
                r2 = ph3_p.tile([128, 2, BQc], F32, tag="prelu2p", name="prelu2p")
                nc.scalar.activation(r2[:], pT[:], mybir.ActivationFunctionType.Relu, scale=-1.0)
                for eh in range(2):
                    nc.vector.tensor_scalar_mul(r2[:, eh, :], r2[:, eh, :], at_sb[:, eh, :])
                nc.vector.tensor_tensor(out=pT[:], in0=r1[:], in1=r2[:],
                                        op=mybir.AluOpType.subtract)
            nc.vector.tensor_copy(out=pTh[:], in_=pT[:])                 # round to bf16
            pThf = ph3_p.tile([128, 2, BQc], F32, tag="pThf")
            nc.vector.tensor_copy(out=pThf[:], in_=pTh[:])               # back to f32
            nc.vector.tensor_tensor(out=pTl[:], in0=pT[:], in1=pThf[:],  # residual, bf16
                                    op=mybir.AluOpType.subtract)
            # packed lhsT for the 32-row tail: [hi(32) | lo(32)] per e-half,
            # so one matmul pass covers both hi and lo contributions
            pPk = ph3_p.tile([128, 2, 64], BF16, tag="pPk")
            nc.vector.tensor_copy(out=pPk[:, :, 0:32], in_=pTh[:, :, 128:BQc])
            nc.vector.tensor_copy(out=pPk[:, :, 32:64], in_=pTl[:, :, 128:BQc])

            # ---- readout y = p @ R^T, bf16 split-3 ----
            rth_ap = d_rth.ap().rearrange("(h p) n -> p h n", p=128)
            rtl_ap = d_rtl.ap().rearrange("(h p) n -> p h n", p=128)
            y_ap = d_y.ap()
            for nt in range(N_TILES):
                n0 = nt * N_TILE
                nsz = min(N_TILE, RO - n0)
                rh = rt_p.tile([128, 2, N_TILE], BF16, tag="rth", name="rth")
                nc.sync.dma_start(rh[:, :, :nsz], rth_ap[:, :, n0:n0 + nsz])
                rl = rt_p.tile([128, 2, N_TILE], BF16, tag="rtl", name="rtl")
                nc.sync.dma_start(rl[:, :, :nsz], rtl_ap[:, :, n0:n0 + nsz])
                # rows 0..127: full 6-pass split-3
                yp = psB.tile([128, N_TILE], F32, tag="yp", bufs=2, space="PSUM")
                ci = 0
                for pt, rt in ((pTh, rh), (pTh, rl), (pTl, rh)):
                    for kh in range(2):
                        nc.tensor.matmul(
                            yp[:, :nsz],
                            lhsT=pt[:, kh, 0:128],
                            rhs=rt[:, kh, :nsz],
                            start=(ci == 0), stop=(ci == 5),
                        )
                        ci += 1
                ysb = ysb_p.tile([128, N_TILE], F32, tag="ysb0", name="ysb0")
                nc.vector.tensor_copy(out=ysb[:, :nsz], in_=yp[:, :nsz])
                nc.sync.dma_start(y_ap[0:128, n0:n0 + nsz], ysb[:, :nsz])
                # rows 128..159: packed hi|lo lhsT vs rh (2 passes) + hi vs rl
                # (2 passes, accumulating into rows 0:32 of the same group)
                yp2 = psB.tile([64, N_TILE], F32, tag="yp2", bufs=2, space="PSUM")
                for kh in range(2):
                    nc.tensor.matmul(yp2[:, :nsz], lhsT=pPk[:, kh, :], rhs=rh[:, kh, :nsz],
                                     start=(kh == 0), stop=(kh == 1))
                yprl = psB.tile([32, N_TILE], F32, tag="yprl", bufs=1, space="PSUM")
                for kh in range(2):
                    nc.tensor.matmul(yprl[:, :nsz], lhsT=pTh[:, kh, 128:BQc],
                                     rhs=rl[:, kh, :nsz], start=(kh == 0), stop=(kh == 1))
                ysb2 = ysb_p.tile([32, N_TILE], F32, tag="ysb1", name="ysb1")
                nc.vector.tensor_copy(out=ysb2[:, :nsz], in_=yp2[0:32, :nsz])
                nc.vector.tensor_tensor(out=ysb2[:, :nsz], in0=ysb2[:, :nsz],
                                        in1=yp2[32:64, :nsz], op=mybir.AluOpType.add)
                nc.vector.tensor_tensor(out=ysb2[:, :nsz], in0=ysb2[:, :nsz],
                                        in1=yprl[:, :nsz], op=mybir.AluOpType.add)
                nc.sync.dma_start(y_ap[128:BQc, n0:n0 + nsz], ysb2[:, :nsz])

    nc.compile()
    return nc


# ------------------------------------------------------------------
# host side
# ------------------------------------------------------------------

_PROG_CACHE = {}


def _get_program(general_mask, general_prelu):
    key = (general_mask, general_prelu)
    if key not in _PROG_CACHE:
        _PROG_CACHE[key] = build_program(*key)
    return _PROG_CACHE[key]


def host_prep(qa_ques, full_rnd, embed, prelu_a, story_mask, query_mask,
              U, V, W, bias, H, R):
    """Build the shared constants and per-core input maps (all numpy)."""
    import ml_dtypes

    qa_ques = np.asarray(qa_ques).astype(np.int32)
    full_rnd = np.asarray(full_rnd).astype(np.int32)
    embed = np.asarray(embed, dtype=np.float32)
    prelu_a = np.asarray(prelu_a, dtype=np.float32)
    story_mask = np.asarray(story_mask, dtype=np.float32)
    query_mask = np.asarray(query_mask, dtype=np.float32)
    U, V, W, bias, H, R = (np.asarray(x, dtype=np.float32) for x in (U, V, W, bias, H, R))

    general_mask = not (np.all(story_mask == 1.0) and np.all(query_mask == 1.0))
    general_prelu = not np.all(prelu_a == 1.0)

    emb = embed.copy()
    emb[0, :] = 0.0  # padding_idx
    keys = emb[-K:]                     # [K, E]
    key_V = keys @ V.T                  # [K, E]
    cb = bias[None, :] + key_V          # [K, E]

    RT = np.ascontiguousarray(R.T)
    RT_hi = RT.astype(ml_dtypes.bfloat16)
    RT_lo = (RT - RT_hi.astype(np.float32)).astype(ml_dtypes.bfloat16)

    common = {
        "emb": emb,
        "rth": RT_hi,
        "rtl": RT_lo,
        "ut": np.ascontiguousarray(U.T),
        "wt": np.ascontiguousarray(W.T),
        "ht": np.ascontiguousarray(H.T),
        "keyst": np.ascontiguousarray(keys.T),
        "cbt": np.ascontiguousarray(cb.T),
    }

    # segment matrices
    hseg = np.zeros((H_TILES, 128, NB), np.float32)
    tok = np.arange(H_TILES * 128)
    valid = tok < NB * L
    hseg[tok[valid] // 128, tok[valid] % 128, tok[valid] // L] = 1.0
    common["hseg"] = hseg

    qseg = np.zeros((Q_TILES, 128, BQc), np.float32)
    tokq = np.arange(Q_TILES * 128)
    qseg[tokq // 128, tokq % 128, tokq // S] = 1.0
    common["qseg"] = qseg

    # additive attention block mask: 0 on the own-b block, -1e30 elsewhere
    amask = np.full((BQc, NB * K), -1e30, np.float32)
    for b in range(NB):
        amask[b * Q:(b + 1) * Q, b * K:(b + 1) * K] = 0.0
    common["amask"] = amask

    if general_mask:
        hmsk = np.zeros((H_TILES, 128, E), np.float32)
        lpos = np.where(valid, tok % L, 0)
        hmsk[tok // 128, tok % 128, :] = story_mask[lpos] * valid[:, None]
        common["hmsk"] = hmsk
        qmsk = np.zeros((5, 128, E), np.float32)
        tq5 = np.arange(5 * 128)
        qmsk[tq5 // 128, tq5 % 128, :] = query_mask[tq5 % S]
        common["qmsk"] = qmsk
    if general_prelu:
        common["at"] = np.ascontiguousarray(prelu_a[:, None])

    in_maps = []
    for c in range(NC):
        bs = slice(c * NB, (c + 1) * NB)
        # hist tokens ordered (r, b, l), padded per round to 512 with idx 0
        fr = full_rnd[bs]                      # [NB, Rn, L]
        idxh = np.zeros((Rn, RND_TOK), np.int32)
        idxh[:, :NB * L] = fr.transpose(1, 0, 2).reshape(Rn, NB * L)
        idxh = idxh.reshape(Rn * H_TILES, 128)
        # query tokens ordered (b, q, s)
        idxq = qa_ques[bs].reshape(Q_TILES, 128)
        m = dict(common)
        m["idxh"] = idxh
        m["idxq"] = idxq
        in_maps.append(m)

    return in_maps, (general_mask, general_prelu)


def kernel(qa_ques, full_rnd, embed, prelu_a, story_mask, query_mask,
           U, V, W, bias, H, R):
    in_maps, flags = host_prep(qa_ques, full_rnd, embed, prelu_a, story_mask,
                               query_mask, U, V, W, bias, H, R)
    nc = _get_program(*flags)
    res = run_bass_kernel_spmd(nc, in_maps, core_ids=list(range(NC)), trace=False)
    y = np.concatenate([res.results[c]["y"] for c in range(NC)], axis=0)
    return y.reshape(B, Q, RO)

